# revision 15
# baseline (speedup 1.0000x reference)
"""GNN SAGEConv (mean-agg) Trainium2 kernel v5, 8-core SPMD.

Architecture (per core, dst-shard of 12500 nodes):
  - x lives in SBUF as 8 bank tables [128, 12504] f32: partition 16g+f =
    feature f (replicated over the 8 Q7 core-groups g); bank b = src nodes
    [12500b, 12500(b+1)); feature 12 = 1.0 (count channel); row 12500 = 0.
  - Host sorts each core's edges by (bank, dst, src) and assigns node
    groups: group g = local nodes [1568g, 1568(g+1)); sections of 392
    nodes.  Per (bank, section): an idx stream (src % 12500) per group,
    column 0 = zero row; widths W[b][s] common across cores/groups (max).
  - Device: ap_gather (8 Q7 cores in parallel) pulls messages
    feature-major [128, W]; tensor_tensor_scan cumsums along the stream;
    a second tiny ap_gather reads the cumsum at each node's last-edge
    position (D); agg[n] = D[n] - D[n-1] accumulated over banks.
  - Epilogue (feature-major): r = 1/max(cnt,1) broadcast via K=1 matmul,
    mean = agg*r, out.T = W_l@mean + W_r@x.T (+bias via ones channel).
  No SWDGE descriptors anywhere; Pool runs only ap_gather ucode.
"""

from contextlib import ExitStack

import numpy as np

N_NODES = 100000
D = 12
NCORES = 8
NCN = 12500          # nodes per core
NGROUP = 8           # Q7 core groups
NG = 1569            # nodes per group (8*1569 = 12552 >= 12500)
NSEC = 3
CSEC = 523           # nodes per section (3*523 = 1569)
CPAD = 528           # D-idx padded per section (mult of 16)
EPW = 512            # epilogue piece width (PSUM bank = 2KB -> <=512 f32)
NBANK = 8
BN = 12500           # src nodes per bank
ZROW = 12500         # zero row index in bank table
TBL = 12504          # bank table elems (>= BN+1, mult of 8)

_MAX_WAITS = 1


def _apply_tile_patches(tile_mod, mybir, vector_clock):
    ScopedClock = vector_clock.ScopedClock

    def _drain_and_barrier(self, tick_clock, wait_clock):
        nc = self.nc
        probe = nc.sync.nop(hint="drain_wait_probe", nofuse=True)
        wait_clock.add_sem_waits(
            probe.ins, ScopedClock({None: tick_clock.global_clock})
        )
        si = probe.ins.sync_info
        waits = list(si.on_wait) if si is not None else []
        if len(waits) > _MAX_WAITS:
            si.on_wait = waits[:_MAX_WAITS]
            for i in range(_MAX_WAITS, len(waits), _MAX_WAITS):
                n = nc.sync.nop(hint="drain_wait_extra", nofuse=True)
                nsi = n.ins.sync_info
                if nsi is None:
                    n.ins.sync_info = mybir.SyncInfo(
                        on_wait=waits[i:i + _MAX_WAITS], on_update=[]
                    )
                else:
                    nsi.on_wait = waits[i:i + _MAX_WAITS]
        nc.sync.drain()
        nc.all_engine_barrier()
        assert self.sems is not None
        popped = nc._tile_sem_poison_stack.pop()
        assert popped is self._sem_poison
        nc.clear_and_free_semaphores(list(self.sems.allocated().values()))
        nc.all_engine_barrier()

    tile_mod.TileContext._drain_and_barrier = _drain_and_barrier


def _split_multi_waits(nc, mybir):
    cnt = 0
    for f in nc.m.functions:
        for bb in f.blocks:
            new = []
            for inst in bb.instructions:
                si = inst.sync_info
                waits = list(si.on_wait) if (si is not None and si.on_wait) else []
                if len(waits) > _MAX_WAITS:
                    extra, keep = waits[:-_MAX_WAITS], waits[-_MAX_WAITS:]
                    for j in range(0, len(extra), _MAX_WAITS):
                        nop = mybir.InstNoOp(name=f"waitsplit_{cnt}", ins=[], outs=[])
                        cnt += 1
                        nop.engine = inst.engine
                        nop.sync_info = mybir.SyncInfo(
                            on_wait=extra[j:j + _MAX_WAITS], on_update=[]
                        )
                        new.append(nop)
                    si.on_wait = keep
                new.append(inst)
            bb.instructions[:] = new


def _wrap16(arr, g):
    """idx list -> [16, len/16] block at partition rows 16g.."""
    n = len(arr)
    assert n % 16 == 0
    return np.ascontiguousarray(np.asarray(arr, np.int16).reshape(n // 16, 16).T)


def build_plan(x, edge_index):
    """Host: per-core edge slicing + idx streams. Returns (Wgrid, cores)."""
    src = np.asarray(edge_index[0], dtype=np.int64)
    dst = np.asarray(edge_index[1], dtype=np.int64)

    cores = []
    for c in range(NCORES):
        m = (dst >= c * NCN) & (dst < (c + 1) * NCN)
        s_c = src[m]
        ld = dst[m] - c * NCN
        b_c = s_c // BN
        order = np.lexsort((s_c, ld, b_c))
        s_c, ld, b_c = s_c[order], ld[order], b_c[order]
        # slice boundaries on (b, node-range) keys
        key = b_c * NCN + ld
        cores.append((s_c, ld, b_c, key))

    # common widths
    Wgrid = np.zeros((NBANK, NSEC), np.int64)
    slices = []  # per core: dict[(g,b,s)] -> (lo, hi)
    for c in range(NCORES):
        s_c, ld, b_c, key = cores[c]
        sl = {}
        for b in range(NBANK):
            for g in range(NGROUP):
                for s in range(NSEC):
                    n0 = g * NG + s * CSEC
                    lo = np.searchsorted(key, b * NCN + n0)
                    hi = np.searchsorted(key, b * NCN + min(n0 + CSEC, NCN))
                    sl[(g, b, s)] = (int(lo), int(hi))
                    Wgrid[b, s] = max(Wgrid[b, s], hi - lo + 1)
        slices.append(sl)
    Wgrid = ((Wgrid + 15) // 16) * 16

    GCOLS = int(Wgrid.sum()) // 16
    DCOLS = NBANK * NSEC * (CPAD // 16)

    in_maps = []
    for c in range(NCORES):
        s_c, ld, b_c, key = cores[c]
        sl = slices[c]
        gi = np.zeros((128, GCOLS), np.int16)
        di = np.zeros((128, DCOLS), np.int16)
        gofs = 0
        dofs = 0
        for b in range(NBANK):
            for s in range(NSEC):
                W = int(Wgrid[b, s])
                for g in range(NGROUP):
                    lo, hi = sl[(g, b, s)]
                    idx = np.full(W, ZROW, np.int16)
                    idx[1:1 + hi - lo] = (s_c[lo:hi] - b * BN).astype(np.int16)
                    gi[16 * g:16 * g + 16, gofs:gofs + W // 16] = _wrap16(idx, g)
                    # D positions: cumsum of per-node counts (1-based stream)
                    n0 = g * NG + s * CSEC
                    cnt = np.bincount(ld[lo:hi] - n0, minlength=CSEC)[:CSEC]
                    ends = np.cumsum(cnt)
                    dd = np.zeros(CPAD, np.int16)
                    dd[:CSEC] = ends.astype(np.int16)
                    di[16 * g:16 * g + 16, dofs:dofs + CPAD // 16] = _wrap16(dd, g)
                gofs += W // 16
                dofs += CPAD // 16
        in_maps.append({"gi": gi, "di": di})
    return Wgrid, in_maps


def build_tables(x, W_l, W_r, bias):
    # channel layout per 16-row group slice: row 0 = ones (count),
    # rows 1..12 = features, rows 13..15 = 0
    x = np.asarray(x, np.float32)
    xb = np.zeros((NBANK, 128, TBL), np.float32)
    for b in range(NBANK):
        blk = x[b * BN:(b + 1) * BN]  # [BN, 12]
        for g in range(NGROUP):
            xb[b, 16 * g, :BN] = 1.0
            xb[b, 16 * g + 1:16 * g + 13, :BN] = blk.T
    wl = np.zeros((16, D), np.float32)
    wr = np.zeros((16, D), np.float32)
    wl[1:13, :] = np.asarray(W_l, np.float32).T
    wr[1:13, :] = np.asarray(W_r, np.float32).T
    wr[0, :] = np.asarray(bias, np.float32)
    return xb, wl, wr


def build_xsT(x, c):
    x = np.asarray(x, np.float32)
    xsT = np.zeros((128, NG), np.float32)
    for g in range(NGROUP):
        n0 = c * NCN + g * NG
        n1 = min(n0 + NG, (c + 1) * NCN)
        w = max(0, n1 - n0)
        if w:
            xsT[16 * g, :w] = 1.0
            xsT[16 * g + 1:16 * g + 13, :w] = x[n0:n1].T
    return xsT


def numpy_run(x, W_l, W_r, bias, edge_index):
    """Simulate the device pipeline exactly (f64 csum) -> full output."""
    Wgrid, in_maps = build_plan(x, edge_index)
    xb, wl, wr = build_tables(x, W_l, W_r, bias)
    out = np.zeros((N_NODES, D), np.float64)
    for c in range(NCORES):
        gi, di = in_maps[c]["gi"], in_maps[c]["di"]
        agg = np.zeros((128, NG))
        gofs = dofs = 0
        for b in range(NBANK):
            for s in range(NSEC):
                W = int(Wgrid[b, s])
                # gather + scan + D-read per group
                for g in range(NGROUP):
                    idx = gi[16 * g:16 * g + 16, gofs:gofs + W // 16].T.reshape(-1)
                    msgs = xb[b, 16 * g:16 * g + 16][:, idx]  # [16, W]
                    cs = np.cumsum(msgs.astype(np.float64), axis=1)
                    dd = di[16 * g:16 * g + 16,
                            dofs:dofs + CPAD // 16].T.reshape(-1)[:CSEC]
                    Dt = cs[:, dd]          # [16, 392]
                    Et = np.concatenate([np.zeros((16, 1)), Dt[:, :-1]], axis=1)
                    agg[16 * g:16 * g + 16, s * CSEC:(s + 1) * CSEC] += Dt - Et
                gofs += W // 16
                dofs += CPAD // 16
        xsT = build_xsT(x, c)
        for g in range(NGROUP):
            a = agg[16 * g:16 * g + 16]     # [16, NG]
            cntv = a[0]
            r = 1.0 / np.maximum(cntv, 1.0)
            mean = a * r[None, :]
            o = (wl.T @ mean + wr.T @ xsT[16 * g:16 * g + 16])
            n0 = c * NCN + g * NG
            n1 = min(n0 + NG, (c + 1) * NCN)
            if n1 > n0:
                out[n0:n1] = o[:, :n1 - n0].T
    return out


def _build_program(Wgrid):
    import concourse.bass as bass
    import concourse.mybir as mybir
    import concourse.tile as tile
    import concourse.vector_clock as vector_clock
    from concourse import library_config
    from concourse.library_overlay import lower_extended_insts

    _apply_tile_patches(tile, mybir, vector_clock)

    f32 = mybir.dt.float32
    i16 = mybir.dt.int16
    AOP = mybir.AluOpType

    GCOLS = int(Wgrid.sum()) // 16
    DCOLS = NBANK * NSEC * (CPAD // 16)
    WMAX = int(Wgrid.max())

    nc = bass.Bass()
    xb = nc.declare_dram_parameter("xb", [NBANK, 128, TBL], f32, isOutput=False)
    gi = nc.declare_dram_parameter("gi", [128, GCOLS], i16, isOutput=False)
    di = nc.declare_dram_parameter("di", [128, DCOLS], i16, isOutput=False)
    xsT = nc.declare_dram_parameter("xsT", [128, NG], f32, isOutput=False)
    wl = nc.declare_dram_parameter("wl", [16, D], f32, isOutput=False)
    wr = nc.declare_dram_parameter("wr", [16, D], f32, isOutput=False)
    out = nc.declare_dram_parameter("out", [12, NGROUP * NG], f32, isOutput=True)

    with ExitStack() as octx:
        tc = octx.enter_context(tile.TileContext(nc))
        keep = octx.enter_context(tc.tile_pool(name="keep", bufs=1))

        nc.gpsimd.load_library(library_config.ap_gather)

        agg = keep.tile([128, NG], f32)
        nc.vector.memset(agg[:], 0.0)
        wl_t = keep.tile([16, D], f32)
        nc.sync.dma_start(out=wl_t[:], in_=wl[:])
        wr_t = keep.tile([16, D], f32)
        nc.sync.dma_start(out=wr_t[:], in_=wr[:])
        ones1 = keep.tile([1, 16], f32)
        nc.vector.memset(ones1[:], 1.0)

        with ExitStack() as pctx:
            xbp = pctx.enter_context(tc.tile_pool(name="xbp", bufs=2))
            gip = pctx.enter_context(tc.tile_pool(name="gip", bufs=4))
            dip = pctx.enter_context(tc.tile_pool(name="dip", bufs=4))
            msp = pctx.enter_context(tc.tile_pool(name="msp", bufs=2))
            csp = pctx.enter_context(tc.tile_pool(name="csp", bufs=3))
            dtp = pctx.enter_context(tc.tile_pool(name="dtp", bufs=2))

            pending = []

            def emit_dg(ent):
                csb, b, s, W = ent
                dit = dip.tile([128, CPAD // 16], i16, tag="di")
                dofs = (b * NSEC + s) * (CPAD // 16)
                nc.sync.dma_start(out=dit[:],
                                  in_=di[:, dofs:dofs + CPAD // 16])
                dt = dtp.tile([128, CPAD], f32, tag="dt")
                nc.gpsimd.ap_gather(
                    out_ap=dt[:].rearrange("p (n d) -> p n d", d=1),
                    in_ap=csb[:, :W].rearrange("p (n d) -> p n d", d=1),
                    idxs_ap=dit[:], channels=128, num_elems=W, d=1,
                    num_idxs=CPAD)
                a_v = agg[:, s * CSEC:(s + 1) * CSEC]
                nc.vector.tensor_tensor(out=a_v, in0=a_v, in1=dt[:, :CSEC],
                                        op=AOP.add)
                a_s = agg[:, s * CSEC + 1:(s + 1) * CSEC]
                nc.vector.tensor_tensor(out=a_s, in0=a_s,
                                        in1=dt[:, :CSEC - 1], op=AOP.subtract)

            gofs = 0
            for b in range(NBANK):
                xbt = xbp.tile([128, TBL], f32, tag="xb")
                nc.sync.dma_start(out=xbt[:], in_=xb[b])
                for s in range(NSEC):
                    W = int(Wgrid[b, s])
                    git = gip.tile([128, WMAX // 16], i16, tag="gi")
                    nc.sync.dma_start(out=git[:, :W // 16],
                                      in_=gi[:, gofs:gofs + W // 16])
                    ms = msp.tile([128, WMAX], f32, tag="ms")
                    nc.gpsimd.ap_gather(
                        out_ap=ms[:, :W].rearrange("p (n d) -> p n d", d=1),
                        in_ap=xbt[:].rearrange("p (n d) -> p n d", d=1),
                        idxs_ap=git[:, :W // 16], channels=128,
                        num_elems=TBL, d=1, num_idxs=W)
                    csb = csp.tile([128, WMAX], f32, tag="cs")
                    nc.vector.tensor_tensor_scan(
                        out=csb[:, :W], data0=ms[:, :W], data1=ms[:, :W],
                        initial=0.0, op0=AOP.add, op1=AOP.bypass)
                    pending.append((csb, b, s, W))
                    if len(pending) > 1:
                        emit_dg(pending.pop(0))
                    gofs += W // 16
            while pending:
                emit_dg(pending.pop(0))

        # ---- epilogue ----
        with ExitStack() as ectx:
            ep = ectx.enter_context(tc.tile_pool(name="ep", bufs=2))
            ps = ectx.enter_context(tc.tile_pool(name="ps", bufs=2,
                                                 space="PSUM"))
            for g in range(NGROUP):
                agt = ep.tile([16, NG], f32, tag="agt")
                nc.sync.dma_start(out=agt[:], in_=agg[16 * g:16 * g + 16, :])
                xst = ep.tile([16, NG], f32, tag="xst")
                nc.sync.dma_start(out=xst[:], in_=xsT[16 * g:16 * g + 16, :])
                # row 0 = cnt; recip over all 16 rows (junk rows harmless)
                rcp = ep.tile([16, NG], f32, tag="rcp")
                nc.vector.tensor_scalar_max(rcp[:], agt[:], 1.0)
                nc.vector.reciprocal(rcp[:], rcp[:])
                for c0 in range(0, NG, EPW):
                    c1 = min(c0 + EPW, NG)
                    w = c1 - c0
                    rb_ps = ps.tile([16, EPW], f32, tag="rb")
                    nc.tensor.matmul(rb_ps[:, :w], ones1[:], rcp[0:1, c0:c1],
                                     start=True, stop=True)
                    rb = ep.tile([16, EPW], f32, tag="rbs")
                    nc.scalar.copy(rb[:, :w], rb_ps[:, :w])
                    mean = ep.tile([16, EPW], f32, tag="mean")
                    nc.vector.tensor_tensor(out=mean[:, :w],
                                            in0=agt[:, c0:c1],
                                            in1=rb[:, :w], op=AOP.mult)
                    o_ps = ps.tile([12, EPW], f32, tag="o")
                    nc.tensor.matmul(o_ps[:, :w], wl_t[:], mean[:, :w],
                                     start=True, stop=False)
                    nc.tensor.matmul(o_ps[:, :w], wr_t[:], xst[:, c0:c1],
                                     start=False, stop=True)
                    ot = ep.tile([12, EPW], f32, tag="ot")
                    nc.vector.tensor_copy(ot[:, :w], o_ps[:, :w])
                    nc.sync.dma_start(out=out[:, g * NG + c0:g * NG + c1],
                                      in_=ot[:, :w])

    _split_multi_waits(nc, mybir)
    lower_extended_insts(nc)
    return nc


def kernel(x, W_l, W_r, b, edge_index):
    import concourse.mybir as mybir  # noqa: F401
    from concourse.bass_utils import run_bass_kernel_spmd

    x = np.asarray(x, dtype=np.float32)
    Wgrid, in_maps = build_plan(x, edge_index)
    xb, wl, wr = build_tables(x, W_l, W_r, b)
    nc = _build_program(Wgrid)

    full_maps = []
    for c in range(NCORES):
        full_maps.append({
            "xb": xb, "gi": in_maps[c]["gi"], "di": in_maps[c]["di"],
            "xsT": build_xsT(x, c), "wl": wl, "wr": wr,
        })

    try:
        res = run_bass_kernel_spmd(
            nc, full_maps, core_ids=list(range(NCORES)), trace=True)
    except ModuleNotFoundError:
        res = run_bass_kernel_spmd(
            nc, full_maps, core_ids=list(range(NCORES)), trace=False)
    if res.exec_time_ns:
        print(f"HW exec time: {res.exec_time_ns} ns")
    if res.instructions_and_trace:
        print("trace path:", res.instructions_and_trace[1])
    if res.profile_json:
        print("profile json:", res.profile_json)

    out = np.empty((N_NODES, D), dtype=np.float32)
    for c in range(NCORES):
        o = res.results[c]["out"]  # [12, NGROUP*NG]
        out[c * NCN:(c + 1) * NCN, :] = o[:, :NCN].T
    return out


# revision 20
# speedup vs baseline: 1.0183x; 1.0183x over previous
"""GNN SAGEConv (mean-agg) Trainium2 kernel v5, 8-core SPMD.

Architecture (per core, dst-shard of 12500 nodes):
  - x lives in SBUF as 8 bank tables [128, 12504] f32: partition 16g+f =
    feature f (replicated over the 8 Q7 core-groups g); bank b = src nodes
    [12500b, 12500(b+1)); feature 12 = 1.0 (count channel); row 12500 = 0.
  - Host sorts each core's edges by (bank, dst, src) and assigns node
    groups: group g = local nodes [1568g, 1568(g+1)); sections of 392
    nodes.  Per (bank, section): an idx stream (src % 12500) per group,
    column 0 = zero row; widths W[b][s] common across cores/groups (max).
  - Device: ap_gather (8 Q7 cores in parallel) pulls messages
    feature-major [128, W]; tensor_tensor_scan cumsums along the stream;
    a second tiny ap_gather reads the cumsum at each node's last-edge
    position (D); agg[n] = D[n] - D[n-1] accumulated over banks.
  - Epilogue (feature-major): r = 1/max(cnt,1) broadcast via K=1 matmul,
    mean = agg*r, out.T = W_l@mean + W_r@x.T (+bias via ones channel).
  No SWDGE descriptors anywhere; Pool runs only ap_gather ucode.
"""

from contextlib import ExitStack

import numpy as np

N_NODES = 100000
D = 12
NCORES = 8
NCN = 12500          # nodes per core
NGROUP = 8           # Q7 core groups
NG = 1569            # nodes per group (8*1569 = 12552 >= 12500)
NSEC = 3
CSEC = 523           # nodes per section (3*523 = 1569)
CPAD = 528           # D-idx padded per section (mult of 16)
EPW = 512            # epilogue piece width (PSUM bank = 2KB -> <=512 f32)
NBANK = 8
BN = 12500           # src nodes per bank
ZROW = 12500         # zero row index in bank table
TBL = 12504          # bank table elems (>= BN+1, mult of 8)

_MAX_WAITS = 1


def _apply_tile_patches(tile_mod, mybir, vector_clock):
    ScopedClock = vector_clock.ScopedClock

    def _drain_and_barrier(self, tick_clock, wait_clock):
        nc = self.nc
        probe = nc.sync.nop(hint="drain_wait_probe", nofuse=True)
        wait_clock.add_sem_waits(
            probe.ins, ScopedClock({None: tick_clock.global_clock})
        )
        si = probe.ins.sync_info
        waits = list(si.on_wait) if si is not None else []
        if len(waits) > _MAX_WAITS:
            si.on_wait = waits[:_MAX_WAITS]
            for i in range(_MAX_WAITS, len(waits), _MAX_WAITS):
                n = nc.sync.nop(hint="drain_wait_extra", nofuse=True)
                nsi = n.ins.sync_info
                if nsi is None:
                    n.ins.sync_info = mybir.SyncInfo(
                        on_wait=waits[i:i + _MAX_WAITS], on_update=[]
                    )
                else:
                    nsi.on_wait = waits[i:i + _MAX_WAITS]
        nc.sync.drain()
        nc.all_engine_barrier()
        assert self.sems is not None
        popped = nc._tile_sem_poison_stack.pop()
        assert popped is self._sem_poison
        nc.clear_and_free_semaphores(list(self.sems.allocated().values()))
        nc.all_engine_barrier()

    tile_mod.TileContext._drain_and_barrier = _drain_and_barrier


def _split_multi_waits(nc, mybir):
    cnt = 0
    for f in nc.m.functions:
        for bb in f.blocks:
            new = []
            for inst in bb.instructions:
                si = inst.sync_info
                waits = list(si.on_wait) if (si is not None and si.on_wait) else []
                if len(waits) > _MAX_WAITS:
                    extra, keep = waits[:-_MAX_WAITS], waits[-_MAX_WAITS:]
                    for j in range(0, len(extra), _MAX_WAITS):
                        nop = mybir.InstNoOp(name=f"waitsplit_{cnt}", ins=[], outs=[])
                        cnt += 1
                        nop.engine = inst.engine
                        nop.sync_info = mybir.SyncInfo(
                            on_wait=extra[j:j + _MAX_WAITS], on_update=[]
                        )
                        new.append(nop)
                    si.on_wait = keep
                new.append(inst)
            bb.instructions[:] = new


def _wrap16(arr, g):
    """idx list -> [16, len/16] block at partition rows 16g.."""
    n = len(arr)
    assert n % 16 == 0
    return np.ascontiguousarray(np.asarray(arr, np.int16).reshape(n // 16, 16).T)


def build_plan(x, edge_index):
    """Host: per-core edge slicing + idx streams. Returns (Wgrid, cores)."""
    src = np.asarray(edge_index[0], dtype=np.int64)
    dst = np.asarray(edge_index[1], dtype=np.int64)

    cores = []
    for c in range(NCORES):
        m = (dst >= c * NCN) & (dst < (c + 1) * NCN)
        s_c = src[m]
        ld = dst[m] - c * NCN
        b_c = s_c // BN
        order = np.lexsort((s_c, ld, b_c))
        s_c, ld, b_c = s_c[order], ld[order], b_c[order]
        # slice boundaries on (b, node-range) keys
        key = b_c * NCN + ld
        cores.append((s_c, ld, b_c, key))

    # common widths
    Wgrid = np.zeros((NBANK, NSEC), np.int64)
    slices = []  # per core: dict[(g,b,s)] -> (lo, hi)
    for c in range(NCORES):
        s_c, ld, b_c, key = cores[c]
        sl = {}
        for b in range(NBANK):
            for g in range(NGROUP):
                for s in range(NSEC):
                    n0 = g * NG + s * CSEC
                    lo = np.searchsorted(key, b * NCN + n0)
                    hi = np.searchsorted(key, b * NCN + min(n0 + CSEC, NCN))
                    sl[(g, b, s)] = (int(lo), int(hi))
                    Wgrid[b, s] = max(Wgrid[b, s], hi - lo + 1)
        slices.append(sl)
    Wgrid = ((Wgrid + 15) // 16) * 16

    GCOLS = int(Wgrid.sum()) // 16
    DCOLS = NBANK * NSEC * (CPAD // 16)

    in_maps = []
    for c in range(NCORES):
        s_c, ld, b_c, key = cores[c]
        sl = slices[c]
        gi = np.zeros((128, GCOLS), np.int16)
        di = np.zeros((128, DCOLS), np.int16)
        gofs = 0
        dofs = 0
        for b in range(NBANK):
            for s in range(NSEC):
                W = int(Wgrid[b, s])
                for g in range(NGROUP):
                    lo, hi = sl[(g, b, s)]
                    idx = np.full(W, ZROW, np.int16)
                    idx[1:1 + hi - lo] = (s_c[lo:hi] - b * BN).astype(np.int16)
                    gi[16 * g:16 * g + 16, gofs:gofs + W // 16] = _wrap16(idx, g)
                    # D positions: cumsum of per-node counts (1-based stream),
                    # offset by the section's base inside the per-bank cs mega
                    sbase = sum(int(Wgrid[b, s2]) for s2 in range(s))
                    n0 = g * NG + s * CSEC
                    cnt = np.bincount(ld[lo:hi] - n0, minlength=CSEC)[:CSEC]
                    ends = np.cumsum(cnt) + sbase
                    dd = np.full(CPAD, sbase, np.int16)
                    dd[:CSEC] = ends.astype(np.int16)
                    di[16 * g:16 * g + 16, dofs:dofs + CPAD // 16] = _wrap16(dd, g)
                gofs += W // 16
                dofs += CPAD // 16
        in_maps.append({"gi": gi, "di": di})
    return Wgrid, in_maps


def build_tables(x, W_l, W_r, bias):
    # channel layout per 16-row group slice: row 0 = ones (count),
    # rows 1..12 = features, rows 13..15 = 0
    x = np.asarray(x, np.float32)
    xb = np.zeros((NBANK, 128, TBL), np.float32)
    for b in range(NBANK):
        blk = x[b * BN:(b + 1) * BN]  # [BN, 12]
        for g in range(NGROUP):
            xb[b, 16 * g, :BN] = 1.0
            xb[b, 16 * g + 1:16 * g + 13, :BN] = blk.T
    wl = np.zeros((16, D), np.float32)
    wr = np.zeros((16, D), np.float32)
    wl[1:13, :] = np.asarray(W_l, np.float32).T
    wr[1:13, :] = np.asarray(W_r, np.float32).T
    wr[0, :] = np.asarray(bias, np.float32)
    return xb, wl, wr


def build_xsT(x, c):
    x = np.asarray(x, np.float32)
    xsT = np.zeros((128, NG), np.float32)
    for g in range(NGROUP):
        n0 = c * NCN + g * NG
        n1 = min(n0 + NG, (c + 1) * NCN)
        w = max(0, n1 - n0)
        if w:
            xsT[16 * g, :w] = 1.0
            xsT[16 * g + 1:16 * g + 13, :w] = x[n0:n1].T
    return xsT


def numpy_run(x, W_l, W_r, bias, edge_index):
    """Simulate the device pipeline exactly (f64 csum) -> full output."""
    Wgrid, in_maps = build_plan(x, edge_index)
    xb, wl, wr = build_tables(x, W_l, W_r, bias)
    out = np.zeros((N_NODES, D), np.float64)
    for c in range(NCORES):
        gi, di = in_maps[c]["gi"], in_maps[c]["di"]
        agg = np.zeros((128, NG))
        gofs = dofs = 0
        for b in range(NBANK):
            for s in range(NSEC):
                W = int(Wgrid[b, s])
                # gather + scan + D-read per group
                sbase = sum(int(Wgrid[b, s2]) for s2 in range(s))
                for g in range(NGROUP):
                    idx = gi[16 * g:16 * g + 16, gofs:gofs + W // 16].T.reshape(-1)
                    msgs = xb[b, 16 * g:16 * g + 16][:, idx]  # [16, W]
                    cs = np.cumsum(msgs.astype(np.float64), axis=1)
                    dd = di[16 * g:16 * g + 16,
                            dofs:dofs + CPAD // 16].T.reshape(-1)[:CSEC] - sbase
                    Dt = cs[:, dd]          # [16, CSEC]
                    Et = np.concatenate([np.zeros((16, 1)), Dt[:, :-1]], axis=1)
                    agg[16 * g:16 * g + 16, s * CSEC:(s + 1) * CSEC] += Dt - Et
                gofs += W // 16
                dofs += CPAD // 16
        xsT = build_xsT(x, c)
        for g in range(NGROUP):
            a = agg[16 * g:16 * g + 16]     # [16, NG]
            cntv = a[0]
            r = 1.0 / np.maximum(cntv, 1.0)
            mean = a * r[None, :]
            o = (wl.T @ mean + wr.T @ xsT[16 * g:16 * g + 16])
            n0 = c * NCN + g * NG
            n1 = min(n0 + NG, (c + 1) * NCN)
            if n1 > n0:
                out[n0:n1] = o[:, :n1 - n0].T
    return out


def _build_program(Wgrid):
    import concourse.bass as bass
    import concourse.mybir as mybir
    import concourse.tile as tile
    import concourse.vector_clock as vector_clock
    from concourse import library_config
    from concourse.library_overlay import lower_extended_insts

    _apply_tile_patches(tile, mybir, vector_clock)

    f32 = mybir.dt.float32
    i16 = mybir.dt.int16
    AOP = mybir.AluOpType

    GCOLS = int(Wgrid.sum()) // 16
    DCOLS = NBANK * NSEC * (CPAD // 16)
    WMAX = int(Wgrid.max())

    nc = bass.Bass()
    xb = nc.declare_dram_parameter("xb", [NBANK, 128, TBL], f32, isOutput=False)
    gi = nc.declare_dram_parameter("gi", [128, GCOLS], i16, isOutput=False)
    di = nc.declare_dram_parameter("di", [128, DCOLS], i16, isOutput=False)
    xsT = nc.declare_dram_parameter("xsT", [128, NG], f32, isOutput=False)
    wl = nc.declare_dram_parameter("wl", [16, D], f32, isOutput=False)
    wr = nc.declare_dram_parameter("wr", [16, D], f32, isOutput=False)
    out = nc.declare_dram_parameter("out", [12, NGROUP * NG], f32, isOutput=True)

    with ExitStack() as octx:
        tc = octx.enter_context(tile.TileContext(nc))
        keep = octx.enter_context(tc.tile_pool(name="keep", bufs=1))

        nc.gpsimd.load_library(library_config.ap_gather)

        agg = keep.tile([128, NG], f32)
        nc.vector.memset(agg[:], 0.0)
        wl_t = keep.tile([16, D], f32)
        nc.sync.dma_start(out=wl_t[:], in_=wl[:])
        wr_t = keep.tile([16, D], f32)
        nc.sync.dma_start(out=wr_t[:], in_=wr[:])
        ones1 = keep.tile([1, 16], f32)
        nc.vector.memset(ones1[:], 1.0)

        with ExitStack() as pctx:
            xbp = pctx.enter_context(tc.tile_pool(name="xbp", bufs=2))
            gip = pctx.enter_context(tc.tile_pool(name="gip", bufs=4))
            dip = pctx.enter_context(tc.tile_pool(name="dip", bufs=4))
            msp = pctx.enter_context(tc.tile_pool(name="msp", bufs=2))
            csp = pctx.enter_context(tc.tile_pool(name="csp", bufs=1))
            dtp = pctx.enter_context(tc.tile_pool(name="dtp", bufs=1))

            pending = []
            bank_w = [sum(int(Wgrid[b, s]) for s in range(NSEC))
                      for b in range(NBANK)]
            CSMW = max(bank_w)

            def emit_dg(ent):
                # one D-extract per bank over the contiguous cs mega-tile
                csb, b, Wtot = ent
                dit = dip.tile([128, NSEC * CPAD // 16], i16, tag="di")
                dofs = b * NSEC * (CPAD // 16)
                nc.sync.dma_start(out=dit[:],
                                  in_=di[:, dofs:dofs + NSEC * CPAD // 16])
                dt = dtp.tile([128, NSEC * CPAD], f32, tag="dt")
                nc.gpsimd.ap_gather(
                    out_ap=dt[:].rearrange("p (n d) -> p n d", d=1),
                    in_ap=csb[:, :Wtot].rearrange("p (n d) -> p n d", d=1),
                    idxs_ap=dit[:], channels=128, num_elems=Wtot, d=1,
                    num_idxs=NSEC * CPAD)
                for s in range(NSEC):
                    a_v = agg[:, s * CSEC:(s + 1) * CSEC]
                    d_v = dt[:, s * CPAD:s * CPAD + CSEC]
                    nc.vector.tensor_tensor(out=a_v, in0=a_v, in1=d_v,
                                            op=AOP.add)
                    a_s = agg[:, s * CSEC + 1:(s + 1) * CSEC]
                    d_s = dt[:, s * CPAD:s * CPAD + CSEC - 1]
                    nc.vector.tensor_tensor(out=a_s, in0=a_s, in1=d_s,
                                            op=AOP.subtract)

            gofs = 0
            for b in range(NBANK):
                xbt = xbp.tile([128, TBL], f32, tag="xb")
                nc.sync.dma_start(out=xbt[:], in_=xb[b])
                csb = csp.tile([128, CSMW], f32, tag="cs")
                sofs = 0
                for s in range(NSEC):
                    W = int(Wgrid[b, s])
                    git = gip.tile([128, WMAX // 16], i16, tag="gi")
                    nc.sync.dma_start(out=git[:, :W // 16],
                                      in_=gi[:, gofs:gofs + W // 16])
                    ms = msp.tile([128, WMAX], f32, tag="ms")
                    nc.gpsimd.ap_gather(
                        out_ap=ms[:, :W].rearrange("p (n d) -> p n d", d=1),
                        in_ap=xbt[:].rearrange("p (n d) -> p n d", d=1),
                        idxs_ap=git[:, :W // 16], channels=128,
                        num_elems=TBL, d=1, num_idxs=W)
                    if s == 0 and pending:
                        # previous bank's D-extract: on Pool after this
                        # bank's first gather, before this bank's scans
                        # overwrite the shared cs mega-tile (WAR)
                        emit_dg(pending.pop(0))
                    nc.vector.tensor_tensor_scan(
                        out=csb[:, sofs:sofs + W], data0=ms[:, :W],
                        data1=ms[:, :W], initial=0.0, op0=AOP.add,
                        op1=AOP.bypass)
                    gofs += W // 16
                    sofs += W
                pending.append((csb, b, sofs))
            while pending:
                emit_dg(pending.pop(0))

        # ---- epilogue ----
        with ExitStack() as ectx:
            ep = ectx.enter_context(tc.tile_pool(name="ep", bufs=2))
            ps = ectx.enter_context(tc.tile_pool(name="ps", bufs=2,
                                                 space="PSUM"))
            for g in range(NGROUP):
                agt = ep.tile([16, NG], f32, tag="agt")
                nc.sync.dma_start(out=agt[:], in_=agg[16 * g:16 * g + 16, :])
                xst = ep.tile([16, NG], f32, tag="xst")
                nc.sync.dma_start(out=xst[:], in_=xsT[16 * g:16 * g + 16, :])
                # row 0 = cnt; recip over all 16 rows (junk rows harmless)
                rcp = ep.tile([16, NG], f32, tag="rcp")
                nc.vector.tensor_scalar_max(rcp[:], agt[:], 1.0)
                nc.vector.reciprocal(rcp[:], rcp[:])
                for c0 in range(0, NG, EPW):
                    c1 = min(c0 + EPW, NG)
                    w = c1 - c0
                    rb_ps = ps.tile([16, EPW], f32, tag="rb")
                    nc.tensor.matmul(rb_ps[:, :w], ones1[:], rcp[0:1, c0:c1],
                                     start=True, stop=True)
                    rb = ep.tile([16, EPW], f32, tag="rbs")
                    nc.scalar.copy(rb[:, :w], rb_ps[:, :w])
                    mean = ep.tile([16, EPW], f32, tag="mean")
                    nc.vector.tensor_tensor(out=mean[:, :w],
                                            in0=agt[:, c0:c1],
                                            in1=rb[:, :w], op=AOP.mult)
                    o_ps = ps.tile([12, EPW], f32, tag="o")
                    nc.tensor.matmul(o_ps[:, :w], wl_t[:], mean[:, :w],
                                     start=True, stop=False)
                    nc.tensor.matmul(o_ps[:, :w], wr_t[:], xst[:, c0:c1],
                                     start=False, stop=True)
                    ot = ep.tile([12, EPW], f32, tag="ot")
                    nc.vector.tensor_copy(ot[:, :w], o_ps[:, :w])
                    nc.sync.dma_start(out=out[:, g * NG + c0:g * NG + c1],
                                      in_=ot[:, :w])

    _split_multi_waits(nc, mybir)
    lower_extended_insts(nc)
    return nc


def kernel(x, W_l, W_r, b, edge_index):
    import concourse.mybir as mybir  # noqa: F401
    from concourse.bass_utils import run_bass_kernel_spmd

    x = np.asarray(x, dtype=np.float32)
    Wgrid, in_maps = build_plan(x, edge_index)
    xb, wl, wr = build_tables(x, W_l, W_r, b)
    nc = _build_program(Wgrid)

    full_maps = []
    for c in range(NCORES):
        full_maps.append({
            "xb": xb, "gi": in_maps[c]["gi"], "di": in_maps[c]["di"],
            "xsT": build_xsT(x, c), "wl": wl, "wr": wr,
        })

    try:
        res = run_bass_kernel_spmd(
            nc, full_maps, core_ids=list(range(NCORES)), trace=True)
    except ModuleNotFoundError:
        res = run_bass_kernel_spmd(
            nc, full_maps, core_ids=list(range(NCORES)), trace=False)
    if res.exec_time_ns:
        print(f"HW exec time: {res.exec_time_ns} ns")
    if res.instructions_and_trace:
        print("trace path:", res.instructions_and_trace[1])
    if res.profile_json:
        print("profile json:", res.profile_json)

    out = np.empty((N_NODES, D), dtype=np.float32)
    for c in range(NCORES):
        o = res.results[c]["out"]  # [12, NGROUP*NG]
        out[c * NCN:(c + 1) * NCN, :] = o[:, :NCN].T
    return out


# revision 21
# speedup vs baseline: 1.0620x; 1.0430x over previous
"""GNN SAGEConv (mean-agg) Trainium2 kernel v5, 8-core SPMD.

Architecture (per core, dst-shard of 12500 nodes):
  - x lives in SBUF as 8 bank tables [128, 12504] f32: partition 16g+f =
    feature f (replicated over the 8 Q7 core-groups g); bank b = src nodes
    [12500b, 12500(b+1)); feature 12 = 1.0 (count channel); row 12500 = 0.
  - Host sorts each core's edges by (bank, dst, src) and assigns node
    groups: group g = local nodes [1569g, 1569(g+1)); sections of 523
    nodes.  Per (bank, section): an idx stream (src % 12500) per group,
    column 0 = zero row; widths W[b][s] common across cores/groups (max).
  - Device: ap_gather (8 Q7 cores in parallel) pulls messages
    feature-major [128, W]; tensor_tensor_scan cumsums each section into
    a per-bank contiguous cs mega-tile; ONE per-bank ap_gather reads the
    cumsum at each node's last-edge position (D, host-offset by section
    base); agg[n] = D[n] - D[n-1] accumulated over banks.
  - Epilogue (feature-major): r = 1/max(cnt,1) broadcast via K=1 matmul,
    mean = agg*r, out.T = W_l@mean + W_r@x.T (+bias via ones channel).
  No SWDGE descriptors anywhere; Pool runs only ap_gather ucode.
"""

from contextlib import ExitStack

import numpy as np

N_NODES = 100000
D = 12
NCORES = 8
NCN = 12500          # nodes per core
NGROUP = 8           # Q7 core groups
NG = 1569            # nodes per group (8*1569 = 12552 >= 12500)
NSEC = 3
CSEC = 523           # nodes per section (3*523 = 1569)
CPAD = 528           # D-idx padded per section (mult of 16)
EPW = 512            # epilogue piece width (PSUM bank = 2KB -> <=512 f32)
NBANK = 8
BN = 12500           # src nodes per bank
ZROW = 12500         # zero row index in bank table
TBL = 12504          # bank table elems (>= BN+1, mult of 8)

_MAX_WAITS = 1


def _apply_tile_patches(tile_mod, mybir, vector_clock):
    ScopedClock = vector_clock.ScopedClock

    def _drain_and_barrier(self, tick_clock, wait_clock):
        nc = self.nc
        probe = nc.sync.nop(hint="drain_wait_probe", nofuse=True)
        wait_clock.add_sem_waits(
            probe.ins, ScopedClock({None: tick_clock.global_clock})
        )
        si = probe.ins.sync_info
        waits = list(si.on_wait) if si is not None else []
        if len(waits) > _MAX_WAITS:
            si.on_wait = waits[:_MAX_WAITS]
            for i in range(_MAX_WAITS, len(waits), _MAX_WAITS):
                n = nc.sync.nop(hint="drain_wait_extra", nofuse=True)
                nsi = n.ins.sync_info
                if nsi is None:
                    n.ins.sync_info = mybir.SyncInfo(
                        on_wait=waits[i:i + _MAX_WAITS], on_update=[]
                    )
                else:
                    nsi.on_wait = waits[i:i + _MAX_WAITS]
        nc.sync.drain()
        nc.all_engine_barrier()
        assert self.sems is not None
        popped = nc._tile_sem_poison_stack.pop()
        assert popped is self._sem_poison
        nc.clear_and_free_semaphores(list(self.sems.allocated().values()))
        nc.all_engine_barrier()

    tile_mod.TileContext._drain_and_barrier = _drain_and_barrier


def _split_multi_waits(nc, mybir):
    cnt = 0
    for f in nc.m.functions:
        for bb in f.blocks:
            new = []
            for inst in bb.instructions:
                si = inst.sync_info
                waits = list(si.on_wait) if (si is not None and si.on_wait) else []
                if len(waits) > _MAX_WAITS:
                    extra, keep = waits[:-_MAX_WAITS], waits[-_MAX_WAITS:]
                    for j in range(0, len(extra), _MAX_WAITS):
                        nop = mybir.InstNoOp(name=f"waitsplit_{cnt}", ins=[], outs=[])
                        cnt += 1
                        nop.engine = inst.engine
                        nop.sync_info = mybir.SyncInfo(
                            on_wait=extra[j:j + _MAX_WAITS], on_update=[]
                        )
                        new.append(nop)
                    si.on_wait = keep
                new.append(inst)
            bb.instructions[:] = new


def _wrap16(arr, g):
    """idx list -> [16, len/16] block at partition rows 16g.."""
    n = len(arr)
    assert n % 16 == 0
    return np.ascontiguousarray(np.asarray(arr, np.int16).reshape(n // 16, 16).T)


def build_plan(x, edge_index):
    """Host: per-core edge slicing + idx streams. Returns (Wgrid, cores)."""
    src = np.asarray(edge_index[0], dtype=np.int64)
    dst = np.asarray(edge_index[1], dtype=np.int64)

    cores = []
    for c in range(NCORES):
        m = (dst >= c * NCN) & (dst < (c + 1) * NCN)
        s_c = src[m]
        ld = dst[m] - c * NCN
        b_c = s_c // BN
        order = np.lexsort((s_c, ld, b_c))
        s_c, ld, b_c = s_c[order], ld[order], b_c[order]
        # slice boundaries on (b, node-range) keys
        key = b_c * NCN + ld
        cores.append((s_c, ld, b_c, key))

    # common widths
    Wgrid = np.zeros((NBANK, NSEC), np.int64)
    slices = []  # per core: dict[(g,b,s)] -> (lo, hi)
    for c in range(NCORES):
        s_c, ld, b_c, key = cores[c]
        sl = {}
        for b in range(NBANK):
            for g in range(NGROUP):
                for s in range(NSEC):
                    n0 = g * NG + s * CSEC
                    lo = np.searchsorted(key, b * NCN + n0)
                    hi = np.searchsorted(key, b * NCN + min(n0 + CSEC, NCN))
                    sl[(g, b, s)] = (int(lo), int(hi))
                    Wgrid[b, s] = max(Wgrid[b, s], hi - lo + 1)
        slices.append(sl)
    Wgrid = ((Wgrid + 15) // 16) * 16

    GCOLS = int(Wgrid.sum()) // 16
    DCOLS = NBANK * NSEC * (CPAD // 16)

    in_maps = []
    for c in range(NCORES):
        s_c, ld, b_c, key = cores[c]
        sl = slices[c]
        gi = np.zeros((128, GCOLS), np.int16)
        di = np.zeros((128, DCOLS), np.int16)
        gofs = 0
        dofs = 0
        for b in range(NBANK):
            for s in range(NSEC):
                W = int(Wgrid[b, s])
                for g in range(NGROUP):
                    lo, hi = sl[(g, b, s)]
                    idx = np.full(W, ZROW, np.int16)
                    idx[1:1 + hi - lo] = (s_c[lo:hi] - b * BN).astype(np.int16)
                    gi[16 * g:16 * g + 16, gofs:gofs + W // 16] = _wrap16(idx, g)
                    # D positions: cumsum of per-node counts (1-based stream),
                    # offset by the section's base inside the per-bank cs mega
                    sbase = sum(int(Wgrid[b, s2]) for s2 in range(s))
                    n0 = g * NG + s * CSEC
                    cnt = np.bincount(ld[lo:hi] - n0, minlength=CSEC)[:CSEC]
                    ends = np.cumsum(cnt) + sbase
                    dd = np.full(CPAD, sbase, np.int16)
                    dd[:CSEC] = ends.astype(np.int16)
                    di[16 * g:16 * g + 16, dofs:dofs + CPAD // 16] = _wrap16(dd, g)
                gofs += W // 16
                dofs += CPAD // 16
        in_maps.append({"gi": gi, "di": di})
    return Wgrid, in_maps


def build_tables(x, W_l, W_r, bias):
    # channel layout per 16-row group slice: row 0 = ones (count),
    # rows 1..12 = features, rows 13..15 = 0
    x = np.asarray(x, np.float32)
    xb = np.zeros((NBANK, 128, TBL), np.float32)
    for b in range(NBANK):
        blk = x[b * BN:(b + 1) * BN]  # [BN, 12]
        for g in range(NGROUP):
            xb[b, 16 * g, :BN] = 1.0
            xb[b, 16 * g + 1:16 * g + 13, :BN] = blk.T
    wl = np.zeros((16, D), np.float32)
    wr = np.zeros((16, D), np.float32)
    wl[1:13, :] = np.asarray(W_l, np.float32).T
    wr[1:13, :] = np.asarray(W_r, np.float32).T
    wr[0, :] = np.asarray(bias, np.float32)
    return xb, wl, wr


def build_xsT(x, c):
    x = np.asarray(x, np.float32)
    xsT = np.zeros((128, NG), np.float32)
    for g in range(NGROUP):
        n0 = c * NCN + g * NG
        n1 = min(n0 + NG, (c + 1) * NCN)
        w = max(0, n1 - n0)
        if w:
            xsT[16 * g, :w] = 1.0
            xsT[16 * g + 1:16 * g + 13, :w] = x[n0:n1].T
    return xsT


def numpy_run(x, W_l, W_r, bias, edge_index):
    """Simulate the device pipeline exactly (f64 csum) -> full output."""
    Wgrid, in_maps = build_plan(x, edge_index)
    xb, wl, wr = build_tables(x, W_l, W_r, bias)
    out = np.zeros((N_NODES, D), np.float64)
    for c in range(NCORES):
        gi, di = in_maps[c]["gi"], in_maps[c]["di"]
        agg = np.zeros((128, NG))
        gofs = dofs = 0
        for b in range(NBANK):
            for s in range(NSEC):
                W = int(Wgrid[b, s])
                # gather + scan + D-read per group
                sbase = sum(int(Wgrid[b, s2]) for s2 in range(s))
                for g in range(NGROUP):
                    idx = gi[16 * g:16 * g + 16, gofs:gofs + W // 16].T.reshape(-1)
                    msgs = xb[b, 16 * g:16 * g + 16][:, idx]  # [16, W]
                    cs = np.cumsum(msgs.astype(np.float64), axis=1)
                    dd = di[16 * g:16 * g + 16,
                            dofs:dofs + CPAD // 16].T.reshape(-1)[:CSEC] - sbase
                    Dt = cs[:, dd]          # [16, CSEC]
                    Et = np.concatenate([np.zeros((16, 1)), Dt[:, :-1]], axis=1)
                    agg[16 * g:16 * g + 16, s * CSEC:(s + 1) * CSEC] += Dt - Et
                gofs += W // 16
                dofs += CPAD // 16
        xsT = build_xsT(x, c)
        for g in range(NGROUP):
            a = agg[16 * g:16 * g + 16]     # [16, NG]
            cntv = a[0]
            r = 1.0 / np.maximum(cntv, 1.0)
            mean = a * r[None, :]
            o = (wl.T @ mean + wr.T @ xsT[16 * g:16 * g + 16])
            n0 = c * NCN + g * NG
            n1 = min(n0 + NG, (c + 1) * NCN)
            if n1 > n0:
                out[n0:n1] = o[:, :n1 - n0].T
    return out


def _build_program(Wgrid):
    import concourse.bass as bass
    import concourse.mybir as mybir
    import concourse.tile as tile
    import concourse.vector_clock as vector_clock
    from concourse import library_config
    from concourse.library_overlay import lower_extended_insts

    _apply_tile_patches(tile, mybir, vector_clock)

    f32 = mybir.dt.float32
    i16 = mybir.dt.int16
    AOP = mybir.AluOpType

    GCOLS = int(Wgrid.sum()) // 16
    DCOLS = NBANK * NSEC * (CPAD // 16)
    WMAX = int(Wgrid.max())

    nc = bass.Bass()
    xb = nc.declare_dram_parameter("xb", [NBANK, 128, TBL], f32, isOutput=False)
    gi = nc.declare_dram_parameter("gi", [128, GCOLS], i16, isOutput=False)
    di = nc.declare_dram_parameter("di", [128, DCOLS], i16, isOutput=False)
    xsT = nc.declare_dram_parameter("xsT", [128, NG], f32, isOutput=False)
    wl = nc.declare_dram_parameter("wl", [16, D], f32, isOutput=False)
    wr = nc.declare_dram_parameter("wr", [16, D], f32, isOutput=False)
    out = nc.declare_dram_parameter("out", [12, NGROUP * NG], f32, isOutput=True)

    with ExitStack() as octx:
        tc = octx.enter_context(tile.TileContext(nc))
        keep = octx.enter_context(tc.tile_pool(name="keep", bufs=1))

        nc.gpsimd.load_library(library_config.ap_gather)

        agg = keep.tile([128, NG], f32)
        nc.vector.memset(agg[:], 0.0)
        wl_t = keep.tile([16, D], f32)
        nc.sync.dma_start(out=wl_t[:], in_=wl[:])
        wr_t = keep.tile([16, D], f32)
        nc.sync.dma_start(out=wr_t[:], in_=wr[:])
        ones1 = keep.tile([1, 16], f32)
        nc.vector.memset(ones1[:], 1.0)

        with ExitStack() as pctx:
            xbp = pctx.enter_context(tc.tile_pool(name="xbp", bufs=2))
            gip = pctx.enter_context(tc.tile_pool(name="gip", bufs=4))
            dip = pctx.enter_context(tc.tile_pool(name="dip", bufs=4))
            msp = pctx.enter_context(tc.tile_pool(name="msp", bufs=2))
            csp = pctx.enter_context(tc.tile_pool(name="csp", bufs=1))
            dtp = pctx.enter_context(tc.tile_pool(name="dtp", bufs=1))

            pending = []
            bank_w = [sum(int(Wgrid[b, s]) for s in range(NSEC))
                      for b in range(NBANK)]
            CSMW = max(bank_w)

            def emit_dg(ent):
                # one D-extract per bank over the contiguous cs mega-tile
                csb, b, Wtot = ent
                dit = dip.tile([128, NSEC * CPAD // 16], i16, tag="di")
                dofs = b * NSEC * (CPAD // 16)
                nc.sync.dma_start(out=dit[:],
                                  in_=di[:, dofs:dofs + NSEC * CPAD // 16])
                dt = dtp.tile([128, NSEC * CPAD], f32, tag="dt")
                nc.gpsimd.ap_gather(
                    out_ap=dt[:].rearrange("p (n d) -> p n d", d=1),
                    in_ap=csb[:, :Wtot].rearrange("p (n d) -> p n d", d=1),
                    idxs_ap=dit[:], channels=128, num_elems=Wtot, d=1,
                    num_idxs=NSEC * CPAD)
                for s in range(NSEC):
                    a_v = agg[:, s * CSEC:(s + 1) * CSEC]
                    d_v = dt[:, s * CPAD:s * CPAD + CSEC]
                    nc.vector.tensor_tensor(out=a_v, in0=a_v, in1=d_v,
                                            op=AOP.add)
                    a_s = agg[:, s * CSEC + 1:(s + 1) * CSEC]
                    d_s = dt[:, s * CPAD:s * CPAD + CSEC - 1]
                    nc.vector.tensor_tensor(out=a_s, in0=a_s, in1=d_s,
                                            op=AOP.subtract)

            gofs = 0
            for b in range(NBANK):
                xbt = xbp.tile([128, TBL], f32, tag="xb")
                nc.sync.dma_start(out=xbt[:], in_=xb[b])
                csb = csp.tile([128, CSMW], f32, tag="cs")
                sofs = 0
                for s in range(NSEC):
                    W = int(Wgrid[b, s])
                    git = gip.tile([128, WMAX // 16], i16, tag="gi")
                    nc.sync.dma_start(out=git[:, :W // 16],
                                      in_=gi[:, gofs:gofs + W // 16])
                    ms = msp.tile([128, WMAX], f32, tag="ms")
                    nc.gpsimd.ap_gather(
                        out_ap=ms[:, :W].rearrange("p (n d) -> p n d", d=1),
                        in_ap=xbt[:].rearrange("p (n d) -> p n d", d=1),
                        idxs_ap=git[:, :W // 16], channels=128,
                        num_elems=TBL, d=1, num_idxs=W)
                    if s == 0 and pending:
                        # previous bank's D-extract: on Pool after this
                        # bank's first gather, before this bank's scans
                        # overwrite the shared cs mega-tile (WAR)
                        emit_dg(pending.pop(0))
                    nc.vector.tensor_tensor_scan(
                        out=csb[:, sofs:sofs + W], data0=ms[:, :W],
                        data1=ms[:, :W], initial=0.0, op0=AOP.add,
                        op1=AOP.bypass)
                    gofs += W // 16
                    sofs += W
                pending.append((csb, b, sofs))
            while pending:
                emit_dg(pending.pop(0))

        # ---- epilogue ----
        with ExitStack() as ectx:
            ep = ectx.enter_context(tc.tile_pool(name="ep", bufs=2))
            ps = ectx.enter_context(tc.tile_pool(name="ps", bufs=2,
                                                 space="PSUM"))
            for g in range(NGROUP):
                agt = ep.tile([16, NG], f32, tag="agt")
                nc.sync.dma_start(out=agt[:], in_=agg[16 * g:16 * g + 16, :])
                xst = ep.tile([16, NG], f32, tag="xst")
                nc.sync.dma_start(out=xst[:], in_=xsT[16 * g:16 * g + 16, :])
                # row 0 = cnt; recip over all 16 rows (junk rows harmless)
                rcp = ep.tile([16, NG], f32, tag="rcp")
                nc.vector.tensor_scalar_max(rcp[:], agt[:], 1.0)
                nc.vector.reciprocal(rcp[:], rcp[:])
                for c0 in range(0, NG, EPW):
                    c1 = min(c0 + EPW, NG)
                    w = c1 - c0
                    rb_ps = ps.tile([16, EPW], f32, tag="rb")
                    nc.tensor.matmul(rb_ps[:, :w], ones1[:], rcp[0:1, c0:c1],
                                     start=True, stop=True)
                    rb = ep.tile([16, EPW], f32, tag="rbs")
                    nc.scalar.copy(rb[:, :w], rb_ps[:, :w])
                    mean = ep.tile([16, EPW], f32, tag="mean")
                    nc.vector.tensor_tensor(out=mean[:, :w],
                                            in0=agt[:, c0:c1],
                                            in1=rb[:, :w], op=AOP.mult)
                    o_ps = ps.tile([12, EPW], f32, tag="o")
                    nc.tensor.matmul(o_ps[:, :w], wl_t[:], mean[:, :w],
                                     start=True, stop=False)
                    nc.tensor.matmul(o_ps[:, :w], wr_t[:], xst[:, c0:c1],
                                     start=False, stop=True)
                    ot = ep.tile([12, EPW], f32, tag="ot")
                    nc.vector.tensor_copy(ot[:, :w], o_ps[:, :w])
                    nc.sync.dma_start(out=out[:, g * NG + c0:g * NG + c1],
                                      in_=ot[:, :w])

    _split_multi_waits(nc, mybir)
    lower_extended_insts(nc)
    return nc


def kernel(x, W_l, W_r, b, edge_index):
    import concourse.mybir as mybir  # noqa: F401
    from concourse.bass_utils import run_bass_kernel_spmd

    x = np.asarray(x, dtype=np.float32)
    Wgrid, in_maps = build_plan(x, edge_index)
    xb, wl, wr = build_tables(x, W_l, W_r, b)
    nc = _build_program(Wgrid)

    full_maps = []
    for c in range(NCORES):
        full_maps.append({
            "xb": xb, "gi": in_maps[c]["gi"], "di": in_maps[c]["di"],
            "xsT": build_xsT(x, c), "wl": wl, "wr": wr,
        })

    try:
        res = run_bass_kernel_spmd(
            nc, full_maps, core_ids=list(range(NCORES)), trace=True)
    except ModuleNotFoundError:
        res = run_bass_kernel_spmd(
            nc, full_maps, core_ids=list(range(NCORES)), trace=False)
    if res.exec_time_ns:
        print(f"HW exec time: {res.exec_time_ns} ns")
    if res.instructions_and_trace:
        print("trace path:", res.instructions_and_trace[1])
    if res.profile_json:
        print("profile json:", res.profile_json)

    out = np.empty((N_NODES, D), dtype=np.float32)
    for c in range(NCORES):
        o = res.results[c]["out"]  # [12, NGROUP*NG]
        out[c * NCN:(c + 1) * NCN, :] = o[:, :NCN].T
    return out


# revision 22
# speedup vs baseline: 1.0700x; 1.0075x over previous
"""GNN SAGEConv (mean-agg) Trainium2 kernel v5, 8-core SPMD.

Architecture (per core, dst-shard of 12500 nodes):
  - x lives in SBUF as 8 bank tables [128, 12504] f32: partition 16g+f =
    feature f (replicated over the 8 Q7 core-groups g); bank b = src nodes
    [12500b, 12500(b+1)); feature 12 = 1.0 (count channel); row 12500 = 0.
  - Host sorts each core's edges by (bank, dst, src) and assigns node
    groups: group g = local nodes [1569g, 1569(g+1)); sections of 523
    nodes.  Per (bank, section): an idx stream (src % 12500) per group,
    column 0 = zero row; widths W[b][s] common across cores/groups (max).
  - Device: ap_gather (8 Q7 cores in parallel) pulls messages
    feature-major [128, W]; tensor_tensor_scan cumsums each section into
    a per-bank contiguous cs mega-tile; ONE per-bank ap_gather reads the
    cumsum at each node's last-edge position (D, host-offset by section
    base); agg[n] = D[n] - D[n-1] accumulated over banks.
  - Epilogue (feature-major): r = 1/max(cnt,1) broadcast via K=1 matmul,
    mean = agg*r, out.T = W_l@mean + W_r@x.T (+bias via ones channel).
  No SWDGE descriptors anywhere; Pool runs only ap_gather ucode.
"""

from contextlib import ExitStack

import numpy as np

N_NODES = 100000
D = 12
NCORES = 8
NCN = 12500          # nodes per core
NGROUP = 8           # Q7 core groups
NG = 1569            # nodes per group (8*1569 = 12552 >= 12500)
NSEC = 3
CSEC = 523           # nodes per section (3*523 = 1569)
CPAD = 528           # D-idx padded per section (mult of 16)
EPW = 512            # epilogue piece width (PSUM bank = 2KB -> <=512 f32)
NBANK = 8
BN = 12500           # src nodes per bank
ZROW = 12500         # zero row index in bank table
TBL = 12504          # bank table elems (>= BN+1, mult of 8)

_MAX_WAITS = 1


def _apply_tile_patches(tile_mod, mybir, vector_clock):
    ScopedClock = vector_clock.ScopedClock

    def _drain_and_barrier(self, tick_clock, wait_clock):
        nc = self.nc
        probe = nc.sync.nop(hint="drain_wait_probe", nofuse=True)
        wait_clock.add_sem_waits(
            probe.ins, ScopedClock({None: tick_clock.global_clock})
        )
        si = probe.ins.sync_info
        waits = list(si.on_wait) if si is not None else []
        if len(waits) > _MAX_WAITS:
            si.on_wait = waits[:_MAX_WAITS]
            for i in range(_MAX_WAITS, len(waits), _MAX_WAITS):
                n = nc.sync.nop(hint="drain_wait_extra", nofuse=True)
                nsi = n.ins.sync_info
                if nsi is None:
                    n.ins.sync_info = mybir.SyncInfo(
                        on_wait=waits[i:i + _MAX_WAITS], on_update=[]
                    )
                else:
                    nsi.on_wait = waits[i:i + _MAX_WAITS]
        nc.sync.drain()
        nc.all_engine_barrier()
        assert self.sems is not None
        popped = nc._tile_sem_poison_stack.pop()
        assert popped is self._sem_poison
        nc.clear_and_free_semaphores(list(self.sems.allocated().values()))
        nc.all_engine_barrier()

    tile_mod.TileContext._drain_and_barrier = _drain_and_barrier


def _split_multi_waits(nc, mybir):
    cnt = 0
    for f in nc.m.functions:
        for bb in f.blocks:
            new = []
            for inst in bb.instructions:
                si = inst.sync_info
                waits = list(si.on_wait) if (si is not None and si.on_wait) else []
                if len(waits) > _MAX_WAITS:
                    extra, keep = waits[:-_MAX_WAITS], waits[-_MAX_WAITS:]
                    for j in range(0, len(extra), _MAX_WAITS):
                        nop = mybir.InstNoOp(name=f"waitsplit_{cnt}", ins=[], outs=[])
                        cnt += 1
                        nop.engine = inst.engine
                        nop.sync_info = mybir.SyncInfo(
                            on_wait=extra[j:j + _MAX_WAITS], on_update=[]
                        )
                        new.append(nop)
                    si.on_wait = keep
                new.append(inst)
            bb.instructions[:] = new


def _wrap16(arr, g):
    """idx list -> [16, len/16] block at partition rows 16g.."""
    n = len(arr)
    assert n % 16 == 0
    return np.ascontiguousarray(np.asarray(arr, np.int16).reshape(n // 16, 16).T)


def build_plan(x, edge_index):
    """Host: per-core edge slicing + idx streams. Returns (Wgrid, cores)."""
    src = np.asarray(edge_index[0], dtype=np.int64)
    dst = np.asarray(edge_index[1], dtype=np.int64)

    cores = []
    for c in range(NCORES):
        m = (dst >= c * NCN) & (dst < (c + 1) * NCN)
        s_c = src[m]
        ld = dst[m] - c * NCN
        b_c = s_c // BN
        order = np.lexsort((s_c, ld, b_c))
        s_c, ld, b_c = s_c[order], ld[order], b_c[order]
        # slice boundaries on (b, node-range) keys
        key = b_c * NCN + ld
        cores.append((s_c, ld, b_c, key))

    # common widths
    Wgrid = np.zeros((NBANK, NSEC), np.int64)
    slices = []  # per core: dict[(g,b,s)] -> (lo, hi)
    for c in range(NCORES):
        s_c, ld, b_c, key = cores[c]
        sl = {}
        for b in range(NBANK):
            for g in range(NGROUP):
                for s in range(NSEC):
                    n0 = g * NG + s * CSEC
                    lo = np.searchsorted(key, b * NCN + n0)
                    hi = np.searchsorted(key, b * NCN + min(n0 + CSEC, NCN))
                    sl[(g, b, s)] = (int(lo), int(hi))
                    Wgrid[b, s] = max(Wgrid[b, s], hi - lo + 1)
        slices.append(sl)
    Wgrid = ((Wgrid + 15) // 16) * 16

    GCOLS = int(Wgrid.sum()) // 16
    DCOLS = NBANK * NSEC * (CPAD // 16)

    in_maps = []
    for c in range(NCORES):
        s_c, ld, b_c, key = cores[c]
        sl = slices[c]
        gi = np.zeros((128, GCOLS), np.int16)
        di = np.zeros((128, DCOLS), np.int16)
        gofs = 0
        dofs = 0
        for b in range(NBANK):
            for s in range(NSEC):
                W = int(Wgrid[b, s])
                for g in range(NGROUP):
                    lo, hi = sl[(g, b, s)]
                    idx = np.full(W, ZROW, np.int16)
                    idx[1:1 + hi - lo] = (s_c[lo:hi] - b * BN).astype(np.int16)
                    gi[16 * g:16 * g + 16, gofs:gofs + W // 16] = _wrap16(idx, g)
                    # D positions: cumsum of per-node counts (1-based stream),
                    # offset by the section's base inside the per-bank cs mega
                    sbase = sum(int(Wgrid[b, s2]) for s2 in range(s))
                    n0 = g * NG + s * CSEC
                    cnt = np.bincount(ld[lo:hi] - n0, minlength=CSEC)[:CSEC]
                    ends = np.cumsum(cnt) + sbase
                    dd = np.full(CPAD, sbase, np.int16)
                    dd[:CSEC] = ends.astype(np.int16)
                    di[16 * g:16 * g + 16, dofs:dofs + CPAD // 16] = _wrap16(dd, g)
                gofs += W // 16
                dofs += CPAD // 16
        in_maps.append({"gi": gi, "di": di})
    return Wgrid, in_maps


def build_tables(x, W_l, W_r, bias):
    # channel layout per 16-row group slice: row 0 = ones (count),
    # rows 1..12 = features, rows 13..15 = 0
    x = np.asarray(x, np.float32)
    xb = np.zeros((NBANK, 128, TBL), np.float32)
    for b in range(NBANK):
        blk = x[b * BN:(b + 1) * BN]  # [BN, 12]
        for g in range(NGROUP):
            xb[b, 16 * g, :BN] = 1.0
            xb[b, 16 * g + 1:16 * g + 13, :BN] = blk.T
    wl = np.zeros((16, D), np.float32)
    wr = np.zeros((16, D), np.float32)
    wl[1:13, :] = np.asarray(W_l, np.float32).T
    wr[1:13, :] = np.asarray(W_r, np.float32).T
    wr[0, :] = np.asarray(bias, np.float32)
    return xb, wl, wr


def build_xsT(x, c):
    x = np.asarray(x, np.float32)
    xsT = np.zeros((128, NG), np.float32)
    for g in range(NGROUP):
        n0 = c * NCN + g * NG
        n1 = min(n0 + NG, (c + 1) * NCN)
        w = max(0, n1 - n0)
        if w:
            xsT[16 * g, :w] = 1.0
            xsT[16 * g + 1:16 * g + 13, :w] = x[n0:n1].T
    return xsT


def numpy_run(x, W_l, W_r, bias, edge_index):
    """Simulate the device pipeline exactly (f64 csum) -> full output."""
    Wgrid, in_maps = build_plan(x, edge_index)
    xb, wl, wr = build_tables(x, W_l, W_r, bias)
    out = np.zeros((N_NODES, D), np.float64)
    for c in range(NCORES):
        gi, di = in_maps[c]["gi"], in_maps[c]["di"]
        agg = np.zeros((128, NG))
        gofs = dofs = 0
        for b in range(NBANK):
            for s in range(NSEC):
                W = int(Wgrid[b, s])
                # gather + scan + D-read per group
                sbase = sum(int(Wgrid[b, s2]) for s2 in range(s))
                for g in range(NGROUP):
                    idx = gi[16 * g:16 * g + 16, gofs:gofs + W // 16].T.reshape(-1)
                    msgs = xb[b, 16 * g:16 * g + 16][:, idx]  # [16, W]
                    cs = np.cumsum(msgs.astype(np.float64), axis=1)
                    dd = di[16 * g:16 * g + 16,
                            dofs:dofs + CPAD // 16].T.reshape(-1)[:CSEC] - sbase
                    Dt = cs[:, dd]          # [16, CSEC]
                    Et = np.concatenate([np.zeros((16, 1)), Dt[:, :-1]], axis=1)
                    agg[16 * g:16 * g + 16, s * CSEC:(s + 1) * CSEC] += Dt - Et
                gofs += W // 16
                dofs += CPAD // 16
        xsT = build_xsT(x, c)
        for g in range(NGROUP):
            a = agg[16 * g:16 * g + 16]     # [16, NG]
            cntv = a[0]
            r = 1.0 / np.maximum(cntv, 1.0)
            mean = a * r[None, :]
            o = (wl.T @ mean + wr.T @ xsT[16 * g:16 * g + 16])
            n0 = c * NCN + g * NG
            n1 = min(n0 + NG, (c + 1) * NCN)
            if n1 > n0:
                out[n0:n1] = o[:, :n1 - n0].T
    return out


def _build_program(Wgrid):
    import concourse.bass as bass
    import concourse.mybir as mybir
    import concourse.tile as tile
    import concourse.vector_clock as vector_clock
    from concourse import library_config
    from concourse.library_overlay import lower_extended_insts

    _apply_tile_patches(tile, mybir, vector_clock)

    f32 = mybir.dt.float32
    i16 = mybir.dt.int16
    AOP = mybir.AluOpType

    GCOLS = int(Wgrid.sum()) // 16
    DCOLS = NBANK * NSEC * (CPAD // 16)
    WMAX = int(Wgrid.max())

    nc = bass.Bass()
    xb = nc.declare_dram_parameter("xb", [NBANK, 128, TBL], f32, isOutput=False)
    gi = nc.declare_dram_parameter("gi", [128, GCOLS], i16, isOutput=False)
    di = nc.declare_dram_parameter("di", [128, DCOLS], i16, isOutput=False)
    xsT = nc.declare_dram_parameter("xsT", [128, NG], f32, isOutput=False)
    wl = nc.declare_dram_parameter("wl", [16, D], f32, isOutput=False)
    wr = nc.declare_dram_parameter("wr", [16, D], f32, isOutput=False)
    out = nc.declare_dram_parameter("out", [12, NGROUP * NG], f32, isOutput=True)

    with ExitStack() as octx:
        tc = octx.enter_context(tile.TileContext(nc))
        keep = octx.enter_context(tc.tile_pool(name="keep", bufs=1))

        nc.gpsimd.load_library(library_config.ap_gather)

        agg = keep.tile([128, NG], f32)
        nc.vector.memset(agg[:], 0.0)
        wl_t = keep.tile([16, D], f32)
        nc.sync.dma_start(out=wl_t[:], in_=wl[:])
        wr_t = keep.tile([16, D], f32)
        nc.sync.dma_start(out=wr_t[:], in_=wr[:])
        ones1 = keep.tile([1, 16], f32)
        nc.vector.memset(ones1[:], 1.0)

        with ExitStack() as pctx:
            xbp = pctx.enter_context(tc.tile_pool(name="xbp", bufs=2))
            gip = pctx.enter_context(tc.tile_pool(name="gip", bufs=4))
            dip = pctx.enter_context(tc.tile_pool(name="dip", bufs=4))
            msp = pctx.enter_context(tc.tile_pool(name="msp", bufs=2))
            csp = pctx.enter_context(tc.tile_pool(name="csp", bufs=1))
            dtp = pctx.enter_context(tc.tile_pool(name="dtp", bufs=1))

            pending = []
            bank_w = [sum(int(Wgrid[b, s]) for s in range(NSEC))
                      for b in range(NBANK)]
            CSMW = max(bank_w)

            def emit_dg(ent):
                # one D-extract per bank over the contiguous cs mega-tile
                csb, b, Wtot = ent
                dit = dip.tile([128, NSEC * CPAD // 16], i16, tag="di")
                dofs = b * NSEC * (CPAD // 16)
                nc.sync.dma_start(out=dit[:],
                                  in_=di[:, dofs:dofs + NSEC * CPAD // 16])
                dt = dtp.tile([128, NSEC * CPAD], f32, tag="dt")
                nc.gpsimd.ap_gather(
                    out_ap=dt[:].rearrange("p (n d) -> p n d", d=1),
                    in_ap=csb[:, :Wtot].rearrange("p (n d) -> p n d", d=1),
                    idxs_ap=dit[:], channels=128, num_elems=Wtot, d=1,
                    num_idxs=NSEC * CPAD)
                for s in range(NSEC):
                    a_v = agg[:, s * CSEC:(s + 1) * CSEC]
                    d_v = dt[:, s * CPAD:s * CPAD + CSEC]
                    nc.vector.tensor_tensor(out=a_v, in0=a_v, in1=d_v,
                                            op=AOP.add)
                    a_s = agg[:, s * CSEC + 1:(s + 1) * CSEC]
                    d_s = dt[:, s * CPAD:s * CPAD + CSEC - 1]
                    nc.vector.tensor_tensor(out=a_s, in0=a_s, in1=d_s,
                                            op=AOP.subtract)

            gofs = 0
            for b in range(NBANK):
                xbt = xbp.tile([128, TBL], f32, tag="xb")
                nc.sync.dma_start(out=xbt[:], in_=xb[b])
                csb = csp.tile([128, CSMW], f32, tag="cs")
                sofs = 0
                for s in range(NSEC):
                    W = int(Wgrid[b, s])
                    git = gip.tile([128, WMAX // 16], i16, tag="gi")
                    nc.sync.dma_start(out=git[:, :W // 16],
                                      in_=gi[:, gofs:gofs + W // 16])
                    ms = msp.tile([128, WMAX], f32, tag="ms")
                    nc.gpsimd.ap_gather(
                        out_ap=ms[:, :W].rearrange("p (n d) -> p n d", d=1),
                        in_ap=xbt[:].rearrange("p (n d) -> p n d", d=1),
                        idxs_ap=git[:, :W // 16], channels=128,
                        num_elems=TBL, d=1, num_idxs=W)
                    if s == 0 and pending:
                        # previous bank's D-extract: on Pool after this
                        # bank's first gather, before this bank's scans
                        # overwrite the shared cs mega-tile (WAR)
                        emit_dg(pending.pop(0))
                    nc.vector.tensor_tensor_scan(
                        out=csb[:, sofs:sofs + W], data0=ms[:, :W],
                        data1=ms[:, :W], initial=0.0, op0=AOP.add,
                        op1=AOP.bypass)
                    gofs += W // 16
                    sofs += W
                pending.append((csb, b, sofs))
            while pending:
                emit_dg(pending.pop(0))

        # ---- epilogue ----
        with ExitStack() as ectx:
            ep = ectx.enter_context(tc.tile_pool(name="ep", bufs=3))
            ps = ectx.enter_context(tc.tile_pool(name="ps", bufs=2,
                                                 space="PSUM"))
            # one full-width recip for all groups (junk rows harmless);
            # row 16g+0 = group g's 1/max(cnt,1)
            rcpf = ep.tile([128, NG], f32, tag="rcpf")
            nc.vector.tensor_scalar_max(rcpf[:], agg[:], 1.0)
            nc.vector.reciprocal(rcpf[:], rcpf[:])
            for g in range(NGROUP):
                agt = ep.tile([16, NG], f32, tag="agt")
                nc.sync.dma_start(out=agt[:], in_=agg[16 * g:16 * g + 16, :])
                xst = ep.tile([16, NG], f32, tag="xst")
                nc.sync.dma_start(out=xst[:], in_=xsT[16 * g:16 * g + 16, :])
                r0 = ep.tile([1, NG], f32, tag="r0")
                nc.sync.dma_start(out=r0[:],
                                  in_=rcpf[16 * g:16 * g + 1, :])
                for c0 in range(0, NG, EPW):
                    c1 = min(c0 + EPW, NG)
                    w = c1 - c0
                    rb_ps = ps.tile([16, EPW], f32, tag="rb")
                    nc.tensor.matmul(rb_ps[:, :w], ones1[:], r0[:, c0:c1],
                                     start=True, stop=True)
                    rb = ep.tile([16, EPW], f32, tag="rbs")
                    nc.scalar.copy(rb[:, :w], rb_ps[:, :w])
                    mean = ep.tile([16, EPW], f32, tag="mean")
                    nc.vector.tensor_tensor(out=mean[:, :w],
                                            in0=agt[:, c0:c1],
                                            in1=rb[:, :w], op=AOP.mult)
                    o_ps = ps.tile([12, EPW], f32, tag="o")
                    nc.tensor.matmul(o_ps[:, :w], wl_t[:], mean[:, :w],
                                     start=True, stop=False)
                    nc.tensor.matmul(o_ps[:, :w], wr_t[:], xst[:, c0:c1],
                                     start=False, stop=True)
                    ot = ep.tile([12, EPW], f32, tag="ot")
                    nc.vector.tensor_copy(ot[:, :w], o_ps[:, :w])
                    nc.sync.dma_start(out=out[:, g * NG + c0:g * NG + c1],
                                      in_=ot[:, :w])

    _split_multi_waits(nc, mybir)
    lower_extended_insts(nc)
    return nc


def kernel(x, W_l, W_r, b, edge_index):
    import concourse.mybir as mybir  # noqa: F401
    from concourse.bass_utils import run_bass_kernel_spmd

    x = np.asarray(x, dtype=np.float32)
    Wgrid, in_maps = build_plan(x, edge_index)
    xb, wl, wr = build_tables(x, W_l, W_r, b)
    nc = _build_program(Wgrid)

    full_maps = []
    for c in range(NCORES):
        full_maps.append({
            "xb": xb, "gi": in_maps[c]["gi"], "di": in_maps[c]["di"],
            "xsT": build_xsT(x, c), "wl": wl, "wr": wr,
        })

    try:
        res = run_bass_kernel_spmd(
            nc, full_maps, core_ids=list(range(NCORES)), trace=True)
    except ModuleNotFoundError:
        res = run_bass_kernel_spmd(
            nc, full_maps, core_ids=list(range(NCORES)), trace=False)
    if res.exec_time_ns:
        print(f"HW exec time: {res.exec_time_ns} ns")
    if res.instructions_and_trace:
        print("trace path:", res.instructions_and_trace[1])
    if res.profile_json:
        print("profile json:", res.profile_json)

    out = np.empty((N_NODES, D), dtype=np.float32)
    for c in range(NCORES):
        o = res.results[c]["out"]  # [12, NGROUP*NG]
        out[c * NCN:(c + 1) * NCN, :] = o[:, :NCN].T
    return out
